# revision 1
# baseline (speedup 1.0000x reference)
"""Bahdanau additive attention on 8 Trainium2 NeuronCores.

  score_t = V^T tanh(W1 value_t + W2 query);  out = softmax(score) @ value

Sharding: data-parallel over batch (16 batches -> 2 per core), weights
replicated.  Per core, value (16 MiB fp32) is read from HBM exactly once
with 8 KiB-line DMAs, cast to bf16 on DVE, transposed with batched
SBUF->SBUF xbar DMA-transposes (one 512 KiB call per supertile), and
consumed by three PE matmul streams (keys^T = W1^T @ value^T; scores =
V^T tanh(keys^T + hidden) with the tanh bias fused on ACT; context =
softmax-weighted value sum) -- all in bf16 with fp32 accumulation.

Hardware quirks this layout works around: the xbar serializes
DMA-transposes against ALL other DMA traffic (hence the strict
loads -> transposes -> scatter DMA ordering, enforced with explicit
dep edges); ACT-ring HWDGE DMAs completion-chain; SWDGE casting DMAs
run at ~8 GB/s/engine; PE transpose-mode does not count as HAM
busy-time (so on-PE transposes keep the array clock at 1.2 GHz).
"""

import functools
import os
import sys

import numpy as np

if "/opt/trn_rl_repo" not in sys.path:
    sys.path.insert(0, "/opt/trn_rl_repo")

B, T, D, U = 16, 8192, 256, 256
NCORES = 8
BPC = B // NCORES          # batches per core
P = 128                    # SBUF partitions
ST = 1024                  # t per supertile
NST = T // ST              # supertiles per batch
CH = 512                   # score/keys chunk width (PSUM bank = 512 fp32)
NCH = ST // CH             # chunks per supertile


@functools.lru_cache(maxsize=1)
def _build():
    from contextlib import ExitStack

    import concourse.bass as bass
    import concourse.tile as tile
    from concourse import bacc, mybir
    from concourse.masks import make_identity

    f32 = mybir.dt.float32
    bf16 = mybir.dt.bfloat16
    Act = mybir.ActivationFunctionType

    nc = bacc.Bacc("TRN2", target_bir_lowering=False, debug=False)

    q = nc.dram_tensor("query", [BPC, D], f32, kind="ExternalInput").ap()
    val = nc.dram_tensor("value", [BPC, T, D], f32, kind="ExternalInput").ap()
    w1 = nc.dram_tensor("W1", [D, U], f32, kind="ExternalInput").ap()
    w2 = nc.dram_tensor("W2", [D, U], f32, kind="ExternalInput").ap()
    vv = nc.dram_tensor("V", [U, 1], f32, kind="ExternalInput").ap()
    out = nc.dram_tensor("out", [BPC, D], f32, kind="ExternalOutput").ap()

    with tile.TileContext(nc) as tc, ExitStack() as ctx:
        consts = ctx.enter_context(tc.tile_pool(name="consts", bufs=1))
        vpool = ctx.enter_context(tc.tile_pool(name="vbf", bufs=1))
        v32pool = ctx.enter_context(tc.tile_pool(name="v32", bufs=6))
        vtpool = ctx.enter_context(tc.tile_pool(name="vt", bufs=4))
        thpool = ctx.enter_context(tc.tile_pool(name="th", bufs=6))
        scpool = ctx.enter_context(tc.tile_pool(name="scsb", bufs=1))
        small = ctx.enter_context(tc.tile_pool(name="small", bufs=1))
        psk = ctx.enter_context(tc.tile_pool(name="psk", bufs=2, space="PSUM"))
        pssc = ctx.enter_context(tc.tile_pool(name="pssc", bufs=2, space="PSUM"))
        psmisc = ctx.enter_context(tc.tile_pool(name="psmisc", bufs=2, space="PSUM"))


        # ---- constants / weights -------------------------------------
        ident = consts.tile([64, 64], f32)
        make_identity(nc, ident)
        ones = consts.tile([P, 1], f32)
        nc.gpsimd.memset(ones, 1.0)

        w1b = consts.tile([P, 2, U], bf16)
        nc.gpsimd.dma_start(out=w1b, in_=w1.rearrange("(kb p) u -> p kb u", p=P))
        w2b = consts.tile([P, 2, U], f32)
        nc.gpsimd.dma_start(out=w2b, in_=w2.rearrange("(kb p) u -> p kb u", p=P))
        vsb = consts.tile([P, 2, 1], bf16)
        nc.gpsimd.dma_start(out=vsb, in_=vv.rearrange("(ub p) o -> p ub o", p=P))

        # hidden = query @ W2, computed as hidden^T [u, b] so it can feed
        # the tanh as a per-partition bias.
        q_sb = consts.tile([BPC, D], f32)
        nc.gpsimd.dma_start(out=q_sb, in_=q)
        qt = consts.tile([P, 2, BPC], f32)
        for kb in range(2):
            psq = psmisc.tile([P, BPC], f32, tag="misc", name="psq")
            nc.tensor.transpose(
                out=psq, in_=q_sb[:, P * kb:P * (kb + 1)], identity=ident[0:BPC, 0:BPC]
            )
            nc.scalar.copy(out=qt[:, kb, :], in_=psq)
        hid = []
        for u in range(2):
            psh = psmisc.tile([P, BPC], f32, tag="misc", name="psh")
            for kb in range(2):
                nc.tensor.matmul(
                    psh,
                    lhsT=w2b[:, kb, P * u:P * (u + 1)],
                    rhs=qt[:, kb, :],
                    start=(kb == 0),
                    stop=(kb == 1),
                )
            h = consts.tile([P, BPC], f32, tag=f"hid{u}")
            nc.scalar.copy(out=h, in_=psh)
            hid.append(h)

        ctx_sb = consts.tile([P, D], f32)

        # ---- value loads (phase 1 of the DMA order) ------------------
        # p-major mapping: VB32[p, f, :] = value[ST*j + 8p + f] gives 8 KiB
        # contiguous DRAM lines per partition.  All loads precede all xbar
        # transposes in DMA order: the hardware serializes DMA-transposes
        # against other DMA traffic, so the clean schedule is
        # loads -> transposes -> scatter DMAs.
        vb = [[None] * NST for _ in range(BPC)]
        vb32s = {}

        def issue_loads(b):
            for j in range(NST):
                VB32 = v32pool.tile([P, ST // P, D], f32, tag="vb32", name="vb32")
                vb32s[(b, j)] = VB32
                nc.sync.dma_start(
                    out=VB32,
                    in_=val[b, ST * j:ST * (j + 1), :].rearrange(
                        "(p f) d -> p f d", f=ST // P
                    ),
                )

        # ---- streaming compute (phase 2: xbar transposes + matmuls) --
        for _b in range(BPC):
            issue_loads(_b)
        sc_sb = [None] * BPC
        s64s = [None] * BPC
        e128s = [None] * BPC
        tr_insts = []
        for b in range(BPC):
            sc_sb[b] = scpool.tile(
                [P, NST, CH], f32, tag=f"scsb{b}", name=f"scsb{b}"
            )
            s64s[b] = [
                small.tile([32, P], f32, tag=f"s64_{b}_{hh}", name=f"s64_{b}_{hh}")
                for hh in range(2)
            ]
            e128s[b] = small.tile([P, 64], bf16, tag=f"e_{b}", name=f"e_{b}")
            for j in range(NST):
                VB32 = vb32s[(b, j)]
                # bf16 cast; each d-half becomes a contiguous 128-run:
                # VB[p, h, f, d'] = value[ST*j + 8p + f, 128h + d']
                VB = vpool.tile([P, 2, ST // P, P], bf16, tag=f"vb_{b}_{j}")
                vb[b][j] = VB
                nc.vector.tensor_copy(
                    out=VB,
                    in_=VB32.rearrange("p f (h d) -> p h f d", h=2),
                )
                # One batched xbar transpose per supertile:
                # VT[d', (h f), t'] = VB[t', h, f, d']  (value^T blocks)
                VT = vtpool.tile([P, 2, ST // P, P], bf16, tag="vt", name="vt")
                tr = nc.sync.dma_start(
                    out=VT.rearrange("p h f t -> p (h f) t"),
                    in_=VB.rearrange("p h f t -> p (h f t)"),
                    transpose=True,
                )
                tr_insts.append(tr)
                psSC = pssc.tile([P, CH], f32)
                ths = []
                for u in range(2):
                    # Two-bank PSUM tile: each chunk's matmul stays inside
                    # one bank, the tanh reads across both in a single op.
                    psK = psk.tile([P, ST], f32, tag="psK", name="psk")
                    for c in range(NCH):
                        for kb in range(2):
                            nc.tensor.matmul(
                                psK[:, CH * c:CH * (c + 1)],
                                lhsT=w1b[:, kb, P * u:P * (u + 1)],
                                rhs=VT[
                                    :, kb, (CH // P) * c:(CH // P) * (c + 1), :
                                ].rearrange("p f t -> p (f t)"),
                                start=(kb == 0),
                                stop=(kb == 1),
                            )
                    th = thpool.tile([P, ST], bf16, tag="th", name="th")
                    nc.scalar.activation(
                        out=th,
                        in_=psK,
                        func=Act.Tanh,
                        bias=hid[u][:, b:b + 1],
                        scale=1.0,
                    )
                    ths.append(th)
                for c in range(NCH):
                    row = 64 * b + 32 * c
                    for u in range(2):
                        nc.tensor.matmul(
                            psSC[row:row + 1, :],
                            lhsT=vsb[:, u, :],
                            rhs=ths[u][:, CH * c:CH * (c + 1)],
                            start=(u == 0),
                            stop=(u == 1),
                            tile_position=(0, row),
                        )
                nc.vector.tensor_copy(out=sc_sb[b][:, j, :], in_=psSC)

        # ---- phase 3: score scatter + softmax + context --------------
        # S64 row r = 8j + 4c + k, so column r of S64^T pairs with VB
        # chunk f = r % 8 of supertile r // 8.  Scatter DMAs are pinned
        # after the last xbar transpose (the hardware serializes
        # transposes against other DMA traffic).
        for b in range(BPC):
            for j in range(NST):
                for c in range(NCH):
                    row = 64 * b + 32 * c
                    rr = 8 * (j % 4) + 4 * c
                    rd = nc.sync.dma_start(
                        out=s64s[b][j // 4][rr:rr + 4, :],
                        in_=sc_sb[b][row:row + 1, j, :].rearrange(
                            "o (k f) -> o k f", k=4
                        ),
                    )
                    tile.add_dep_helper(
                        rd.ins, tr_insts[-1].ins, sync=True,
                        reason="keep scatter DMAs after all xbar transposes",
                    )
        psCs = []
        invSs = []
        for b in range(BPC):
            for hh in range(2):
                psTS = psmisc.tile([P, 32], f32, tag="misc", name="psts")
                nc.tensor.transpose(
                    out=psTS,
                    in_=s64s[b][hh],
                    identity=ident[0:32, 0:32],
                )
                nc.scalar.activation(
                    out=e128s[b][:, 32 * hh:32 * (hh + 1)],
                    in_=psTS,
                    func=Act.Exp,
                    scale=1.0,
                )
            pb = small.tile([P, 1], f32, tag=f"pb_{b}", name=f"pb_{b}")
            nc.vector.reduce_sum(out=pb, in_=e128s[b], axis=mybir.AxisListType.X)
            psS = psmisc.tile([P, 1], f32, tag="misc", name="pss")
            nc.tensor.matmul(
                psS[32 * b:32 * b + 1, :], lhsT=ones, rhs=pb, start=True, stop=True
            )
            invS = small.tile([P, 1], f32, tag=f"invs_{b}", name=f"invs_{b}")
            nc.vector.reciprocal(
                out=invS[32 * b:32 * b + 1, :], in_=psS[32 * b:32 * b + 1, :]
            )
            invSs.append(invS)
            psCs.append(psk.tile([P, D], f32, tag="psK", name=f"psc_{b}"))
        # Interleave the two batches' context accumulations so adjacent
        # M=1 matmuls land on different PSUM column groups (0 / 32) and
        # run concurrently on the PE array.
        for r in range(64):
            for b in range(BPC):
                nc.tensor.matmul(
                    psCs[b][32 * b:32 * b + 1, :],
                    lhsT=e128s[b][:, r:r + 1],
                    rhs=vb[b][r // 8][:, :, r % 8, :],
                    start=(r == 0),
                    stop=(r == 63),
                )
        for b in range(BPC):
            nc.vector.tensor_scalar_mul(
                out=ctx_sb[32 * b:32 * b + 1, :],
                in0=psCs[b][32 * b:32 * b + 1, :],
                scalar1=invSs[b][32 * b:32 * b + 1, :],
            )
            nc.sync.dma_start(out=out[b:b + 1, :], in_=ctx_sb[32 * b:32 * b + 1, :])

    nc.finalize()
    return nc
def _run(inputs, trace=False):
    from concourse import bass_utils

    nc = _build()
    in_maps = [
        {
            "query": np.ascontiguousarray(inputs["query"][BPC * i:BPC * (i + 1)]),
            "value": np.ascontiguousarray(inputs["value"][BPC * i:BPC * (i + 1)]),
            "W1": np.asarray(inputs["W1"]),
            "W2": np.asarray(inputs["W2"]),
            "V": np.asarray(inputs["V"]),
        }
        for i in range(NCORES)
    ]
    res = bass_utils.run_bass_kernel_spmd(
        nc, in_maps, core_ids=list(range(NCORES)), trace=trace
    )
    outp = np.concatenate([r["out"] for r in res.results], axis=0)
    return outp.astype(np.float32), res


def kernel(**inputs) -> np.ndarray:
    outp, _ = _run(inputs, trace=False)
    return outp

